# revision 1
# baseline (speedup 1.0000x reference)
"""KAN layer (polynomial basis) TRN2 kernel.

out = gelu(sum_{i,k} x[b,i]^k * W[i,k,j] + bias[j]),  exact gelu.
B=4096, D=1024, K=5, U=1024, fp32 I/O.

Strategy:
  - Data-parallel over batch: 8 cores x 512 rows each.
  - k=0 term (x^0=1) constant-folded on host into the bias:
    bias_total = bias + sum_i W[i,0,:].
  - x is fed pre-transposed ([D, B_local]) so the contraction dim (D)
    lands on SBUF partitions; powers x^2,x^3,x^4 computed on-device (DVE).
  - Split-precision matmuls: every operand v = vh + vl with vh,vl bf16
    (16 mantissa bits total). out ~= xh@wh + xh@wl + xl@wh per term ->
    ~4e-6 relative error (fp32-class) at 3 bf16 matmuls per fp32 matmul
    (bf16 MM = 1 cyc/row vs fp32 = 4 cyc/row on TRN2 PE).
  - W hi/lo split + tiling done host-side (weights are pure inputs);
    x-power splits on device.
  - Output computed transposed ([U, B_local]) so the per-unit bias is a
    per-partition scalar, fused into the final Gelu activation; host
    transposes back during the gather.
"""

import os
import numpy as np
import ml_dtypes

from concourse import bacc
import concourse.mybir as mybir
import concourse.tile as tile
from concourse.bass_utils import run_bass_kernel_spmd

F32 = mybir.dt.float32
BF16 = mybir.dt.bfloat16
AF = mybir.ActivationFunctionType

NCORES = 8
B, D, K, U = 4096, 1024, 5, 1024
BL = B // NCORES  # 512 batch rows per core
ND = D // 128  # 8 d chunks
NU = U // 128  # 8 u chunks

LAST_EXEC_TIME_NS = None


def _build():
    nc = bacc.Bacc("TRN2", target_bir_lowering=False, debug=False)
    xt = nc.dram_tensor("xt", [D, BL], F32, kind="ExternalInput").ap()
    wblob = nc.dram_tensor(
        "wblob", [NU, ND, 128, 4 * 2 * 128], BF16, kind="ExternalInput"
    ).ap()
    bias2d = nc.dram_tensor("bias2d", [128, NU], F32, kind="ExternalInput").ap()
    out_t = nc.dram_tensor("out_t", [U, BL], F32, kind="ExternalOutput").ap()

    with tile.TileContext(nc) as tc:
        with (
            tc.tile_pool(name="xres", bufs=1) as xres,
            tc.tile_pool(name="tmp", bufs=2) as tmp,
            tc.tile_pool(name="wp", bufs=4) as wp,
            tc.tile_pool(name="op", bufs=2) as op,
            tc.tile_pool(name="ps", bufs=2, space="PSUM") as ps,
        ):
            bias_sb = xres.tile([128, NU], F32, name="bias_sb")
            nc.sync.dma_start(bias_sb, bias2d)

            # ---- powers + hi/lo splits, per d chunk (all resident) ----
            H = [[None] * ND for _ in range(4)]  # H[k][d], k: x^1..x^4
            L = [[None] * ND for _ in range(4)]
            for d in range(ND):
                xf = xres.tile([128, BL], F32, name=f"xf_{d}")
                nc.sync.dma_start(xf, xt[d * 128 : (d + 1) * 128, :])
                x2f = tmp.tile([128, BL], F32, name="x2f", tag="x2f")
                nc.vector.tensor_mul(out=x2f, in0=xf, in1=xf)
                x3f = tmp.tile([128, BL], F32, name="x3f", tag="x3f")
                nc.vector.tensor_mul(out=x3f, in0=x2f, in1=xf)
                x4f = tmp.tile([128, BL], F32, name="x4f", tag="x4f")
                nc.vector.tensor_mul(out=x4f, in0=x2f, in1=x2f)
                for k, src in enumerate([xf, x2f, x3f, x4f]):
                    h = xres.tile([128, BL], BF16, name=f"h{k}_{d}")
                    nc.vector.tensor_copy(h, src)
                    l = xres.tile([128, BL], BF16, name=f"l{k}_{d}")
                    nc.vector.tensor_sub(out=l, in0=src, in1=h)
                    H[k][d] = h
                    L[k][d] = l

            # ---- matmuls: out_T[u,:] = sum_{d,k} W[d,k,u].T @ x^k_T[d,:] ----
            for u in range(NU):
                pacc = ps.tile([128, BL], F32, name="pacc", tag="pacc")
                for d in range(ND):
                    wt = wp.tile([128, 4 * 2 * 128], BF16, name="wt", tag="wt")
                    nc.sync.dma_start(wt, wblob[u, d])
                    for k in range(4):
                        wh = wt[:, k * 256 : k * 256 + 128]
                        wl = wt[:, k * 256 + 128 : k * 256 + 256]
                        nc.tensor.matmul(
                            pacc, wh, H[k][d],
                            start=(d == 0 and k == 0), stop=False,
                        )
                        nc.tensor.matmul(pacc, wh, L[k][d], start=False, stop=False)
                        nc.tensor.matmul(
                            pacc, wl, H[k][d],
                            start=False, stop=(d == ND - 1 and k == 3),
                        )
                osb = op.tile([128, BL], F32, name="osb", tag="osb")
                nc.scalar.activation(
                    osb, pacc, AF.Gelu, bias=bias_sb[:, u : u + 1], scale=1.0
                )
                nc.sync.dma_start(out_t[u * 128 : (u + 1) * 128, :], osb)

    nc.compile()
    return nc


_NC_CACHE = None


def kernel(x, basis_weights, bias):
    global _NC_CACHE, LAST_EXEC_TIME_NS
    x = np.asarray(x, dtype=np.float32)
    W = np.asarray(basis_weights, dtype=np.float32)
    bias = np.asarray(bias, dtype=np.float32)

    # ---- host prep (layout only + constant folding of the x^0 term) ----
    xT = np.ascontiguousarray(x.T)  # (D, B)
    Wk = W[:, 1:5, :]  # (D, 4, U)
    wh = Wk.astype(ml_dtypes.bfloat16)
    wl = (Wk - wh.astype(np.float32)).astype(ml_dtypes.bfloat16)
    st = np.stack([wh, wl], axis=2)  # (D, 4, 2, U)
    blob = st.reshape(ND, 128, 4, 2, NU, 128).transpose(4, 0, 1, 2, 3, 5)
    blob = np.ascontiguousarray(blob.reshape(NU, ND, 128, 4 * 2 * 128))
    bias_total = (
        bias.astype(np.float64) + W[:, 0, :].astype(np.float64).sum(axis=0)
    ).astype(np.float32)
    bias2d = np.ascontiguousarray(bias_total.reshape(NU, 128).T)

    in_maps = []
    for i in range(NCORES):
        xt_i = np.ascontiguousarray(xT[:, i * BL : (i + 1) * BL])
        in_maps.append({"xt": xt_i, "wblob": blob, "bias2d": bias2d})

    if _NC_CACHE is None:
        _NC_CACHE = _build()
    nc = _NC_CACHE

    trace = bool(os.environ.get("KERNEL_TRACE"))
    res = run_bass_kernel_spmd(
        nc, in_maps, core_ids=list(range(NCORES)), trace=trace
    )
    LAST_EXEC_TIME_NS = res.exec_time_ns

    out = np.empty((B, U), dtype=np.float32)
    for i in range(NCORES):
        out[i * BL : (i + 1) * BL, :] = res.results[i]["out_t"].T
    return out



# revision 2
# speedup vs baseline: 2.5433x; 2.5433x over previous
"""KAN layer (polynomial basis) TRN2 kernel.

out = gelu(sum_{i,k} x[b,i]^k * W[i,k,j] + bias[j]),  exact gelu.
B=4096, D=1024, K=5, U=1024, fp32 I/O.

Strategy:
  - Data-parallel over batch: 8 cores x 512 rows each.
  - k=0 term (x^0=1) constant-folded on host into the bias:
    bias_total = bias + sum_i W[i,0,:].
  - Plain bf16 matmuls (x^k and W rounded once to bf16, fp32 PSUM
    accumulate): offline sim shows rel err ~3.9e-3 vs the 2e-2 gate.
    256 matmuls/core at the hardware's 216ns/MM N=512 streaming rate.
  - Loop structure d-outer / (k,u)-inner with all 8 PSUM banks as
    per-u accumulators: matmuls start as soon as chunk 0's powers
    exist (kills the serial prologue), and the last d-chunk runs
    u-major so activations + output DMAs pipeline with the final MMs.
  - Powers computed per d-chunk in fp32: x^2 and x^4 on the Scalar
    engine (Square activation), bf16 casts + x^3 on DVE.
  - Output computed transposed ([U, B_local]) so the per-unit bias is a
    per-partition scalar, fused into the final Gelu activation; host
    transposes back during the gather.
"""

import os
import numpy as np
import ml_dtypes

from concourse import bacc
import concourse.mybir as mybir
import concourse.tile as tile
from concourse.bass_utils import run_bass_kernel_spmd

F32 = mybir.dt.float32
BF16 = mybir.dt.bfloat16
AF = mybir.ActivationFunctionType

NCORES = 8
B, D, K, U = 4096, 1024, 5, 1024
BL = B // NCORES  # 512 batch rows per core
ND = D // 128  # 8 d chunks
NU = U // 128  # 8 u chunks

LAST_EXEC_TIME_NS = None


def _build():
    nc = bacc.Bacc("TRN2", target_bir_lowering=False, debug=False)
    xt = nc.dram_tensor("xt", [D, BL], F32, kind="ExternalInput").ap()
    # wblob[d, k, p, u*128+m] = W[d*128+p, k+1, u*128+m]
    wblob = nc.dram_tensor(
        "wblob", [ND, 4, 128, NU * 128], BF16, kind="ExternalInput"
    ).ap()
    bias2d = nc.dram_tensor("bias2d", [128, NU], F32, kind="ExternalInput").ap()
    out_t = nc.dram_tensor("out_t", [U, BL], F32, kind="ExternalOutput").ap()

    with tile.TileContext(nc) as tc:
        with (
            tc.tile_pool(name="const", bufs=1) as constp,
            tc.tile_pool(name="xp", bufs=2) as xp,
            tc.tile_pool(name="pw", bufs=2) as pw,
            tc.tile_pool(name="wp", bufs=3) as wp,
            tc.tile_pool(name="op", bufs=2) as op,
            tc.tile_pool(name="ps", bufs=1, space="PSUM") as ps,
        ):
            bias_sb = constp.tile([128, NU], F32, name="bias_sb")
            nc.sync.dma_start(bias_sb, bias2d)

            # 8 persistent PSUM accumulators, one bank per u chunk.
            pacc = [ps.tile([128, BL], F32, name=f"pacc{u}") for u in range(NU)]

            for d in range(ND):
                xf = xp.tile([128, BL], F32, name="xf", tag="xf")
                nc.sync.dma_start(xf, xt[d * 128 : (d + 1) * 128, :])

                # powers of x for this chunk, bf16, rounded from fp32
                x1b = pw.tile([128, BL], BF16, name="x1b", tag="x1b")
                nc.vector.tensor_copy(x1b, xf)
                x2f = xp.tile([128, BL], F32, name="x2f", tag="x2f")
                nc.scalar.activation(x2f, xf, AF.Square)
                x2b = pw.tile([128, BL], BF16, name="x2b", tag="x2b")
                nc.vector.tensor_copy(x2b, x2f)
                x3b = pw.tile([128, BL], BF16, name="x3b", tag="x3b")
                nc.vector.tensor_mul(out=x3b, in0=x2f, in1=xf)
                x4b = pw.tile([128, BL], BF16, name="x4b", tag="x4b")
                nc.scalar.activation(x4b, x2f, AF.Square)
                H = [x1b, x2b, x3b, x4b]

                wk = []
                for k in range(4):
                    wt = wp.tile([128, NU * 128], BF16, name="wt", tag=f"wt{k}")
                    nc.sync.dma_start(wt, wblob[d, k])
                    wk.append(wt)

                if d < ND - 1:
                    for k in range(4):
                        for u in range(NU):
                            nc.tensor.matmul(
                                pacc[u],
                                wk[k][:, u * 128 : (u + 1) * 128],
                                H[k],
                                start=(d == 0 and k == 0),
                                stop=False,
                            )
                else:
                    # final chunk u-major: each u finishes early so its
                    # Gelu + output DMA overlap the remaining matmuls
                    for u in range(NU):
                        for k in range(4):
                            nc.tensor.matmul(
                                pacc[u],
                                wk[k][:, u * 128 : (u + 1) * 128],
                                H[k],
                                start=False,
                                stop=(k == 3),
                            )
                        osb = op.tile([128, BL], F32, name="osb", tag="osb")
                        nc.scalar.activation(
                            osb, pacc[u], AF.Gelu, bias=bias_sb[:, u : u + 1]
                        )
                        nc.sync.dma_start(out_t[u * 128 : (u + 1) * 128, :], osb)

    nc.compile()
    return nc


_NC_CACHE = None


def kernel(x, basis_weights, bias):
    global _NC_CACHE, LAST_EXEC_TIME_NS
    x = np.asarray(x, dtype=np.float32)
    W = np.asarray(basis_weights, dtype=np.float32)
    bias = np.asarray(bias, dtype=np.float32)

    # ---- host prep (layout only + constant folding of the x^0 term) ----
    xT = np.ascontiguousarray(x.T)  # (D, B)
    Wk = W[:, 1:5, :]  # (D, 4, U)
    blob = np.ascontiguousarray(
        Wk.reshape(ND, 128, 4, NU * 128).transpose(0, 2, 1, 3)
    ).astype(ml_dtypes.bfloat16)  # [ND, 4, 128, NU*128]
    bias_total = (
        bias.astype(np.float64) + W[:, 0, :].astype(np.float64).sum(axis=0)
    ).astype(np.float32)
    bias2d = np.ascontiguousarray(bias_total.reshape(NU, 128).T)

    in_maps = []
    for i in range(NCORES):
        xt_i = np.ascontiguousarray(xT[:, i * BL : (i + 1) * BL])
        in_maps.append({"xt": xt_i, "wblob": blob, "bias2d": bias2d})

    if _NC_CACHE is None:
        _NC_CACHE = _build()
    nc = _NC_CACHE

    trace = bool(os.environ.get("KERNEL_TRACE"))
    res = run_bass_kernel_spmd(
        nc, in_maps, core_ids=list(range(NCORES)), trace=trace
    )
    LAST_EXEC_TIME_NS = res.exec_time_ns

    out = np.empty((B, U), dtype=np.float32)
    for i in range(NCORES):
        out[i * BL : (i + 1) * BL, :] = res.results[i]["out_t"].T
    return out


# revision 3
# speedup vs baseline: 2.9427x; 1.1570x over previous
"""KAN layer (polynomial basis) TRN2 kernel.

out = gelu(sum_{i,k} x[b,i]^k * W[i,k,j] + bias[j]),  exact gelu.
B=4096, D=1024, K=5, U=1024, fp32 I/O.

Strategy:
  - Data-parallel over batch: 8 cores x 512 rows each.
  - k=0 term (x^0=1) constant-folded on host into the bias.
  - Mixed precision chosen against the 2e-2 rel-err gate (offline sim:
    rel ~5.3e-3): k=1,2 terms in fp8e4 with DoubleRow perf mode (2
    contraction chunks per MM at 0.5 cyc/row), k=3,4 terms in bf16.
    Power-of-2 scales keep products exact: x*(1/4) vs W1*4, x^2*(1/16)
    vs W2*16.  All powers computed in fp32, rounded once.
  - Loop d-chunk-pair-outer / u-inner with all 8 PSUM banks as per-u
    accumulators; the last pair runs u-major so Gelu + output DMA
    pipeline with the final matmuls.
  - x/bias/out DMAs on the GpSimd queue, weight DMAs on the Sync
    queue (parallel issue), powers split across Scalar (Square) and
    DVE engines.
  - Output computed transposed ([U, B_local]) so the per-unit bias is a
    per-partition scalar, fused into the final Gelu activation; host
    transposes back during the gather.
"""

import os
import numpy as np
import ml_dtypes

from concourse import bacc
import concourse.mybir as mybir
import concourse.tile as tile
from concourse.bass_utils import run_bass_kernel_spmd

F32 = mybir.dt.float32
BF16 = mybir.dt.bfloat16
FP8 = mybir.dt.float8e4
AF = mybir.ActivationFunctionType
DR = mybir.MatmulPerfMode.DoubleRow

NCORES = 8
B, D, K, U = 4096, 1024, 5, 1024
BL = B // NCORES  # 512 batch rows per core
ND = D // 128  # 8 d chunks
NP = ND // 2  # 4 d-chunk pairs
NU = U // 128  # 8 u chunks

XS1 = 0.25      # x^1 fp8 pre-scale (W1 scaled by 1/XS1)
XS2 = 0.0625    # x^2 fp8 pre-scale (W2 scaled by 1/XS2)

LAST_EXEC_TIME_NS = None


def _build():
    nc = bacc.Bacc("TRN2", target_bir_lowering=False, debug=False)
    xt = nc.dram_tensor("xt", [D, BL], F32, kind="ExternalInput").ap()
    # wb16[d, kk, p, m] = W[d*128+p, kk+3, m]
    wb16 = nc.dram_tensor(
        "wb16", [ND, 2, 128, NU * 128], BF16, kind="ExternalInput"
    ).ap()
    # wf8[j, kk, p, c, m] = W[(2j+c)*128+p, kk+1, m] / XS_{kk+1}
    wf8 = nc.dram_tensor(
        "wf8", [NP, 2, 128, 2, NU * 128], FP8, kind="ExternalInput"
    ).ap()
    bias2d = nc.dram_tensor("bias2d", [128, NU], F32, kind="ExternalInput").ap()
    out_t = nc.dram_tensor("out_t", [U, BL], F32, kind="ExternalOutput").ap()

    with tile.TileContext(nc) as tc:
        with (
            tc.tile_pool(name="const", bufs=1) as constp,
            tc.tile_pool(name="xp", bufs=3) as xp,
            tc.tile_pool(name="pw", bufs=3) as pw,
            tc.tile_pool(name="qp", bufs=2) as qp,
            tc.tile_pool(name="wp", bufs=3) as wp,
            tc.tile_pool(name="wq", bufs=2) as wq,
            tc.tile_pool(name="op", bufs=4) as op,
            tc.tile_pool(name="ps", bufs=1, space="PSUM") as ps,
        ):
            bias_sb = constp.tile([128, NU], F32, name="bias_sb")
            nc.gpsimd.dma_start(bias_sb, bias2d)

            # 8 persistent PSUM accumulators, one bank per u chunk.
            pacc = [ps.tile([128, BL], F32, name=f"pacc{u}") for u in range(NU)]

            for j in range(NP):
                x1q = qp.tile([128, 2, BL], FP8, name="x1q", tag="x1q")
                x2q = qp.tile([128, 2, BL], FP8, name="x2q", tag="x2q")
                H34 = []  # [c][kk] bf16 tiles
                for c in range(2):
                    d = 2 * j + c
                    xf = xp.tile([128, BL], F32, name="xf", tag="xf")
                    nc.gpsimd.dma_start(xf, xt[d * 128 : (d + 1) * 128, :])
                    x2f = xp.tile([128, BL], F32, name="x2f", tag="x2f")
                    nc.scalar.activation(x2f, xf, AF.Square)
                    nc.vector.tensor_scalar_mul(x1q[:, c], xf, XS1)
                    nc.scalar.activation(x2q[:, c], xf, AF.Square, scale=0.25)
                    x3b = pw.tile([128, BL], BF16, name="x3b", tag="x3b")
                    nc.vector.tensor_mul(out=x3b, in0=x2f, in1=xf)
                    x4b = pw.tile([128, BL], BF16, name="x4b", tag="x4b")
                    nc.scalar.activation(x4b, x2f, AF.Square)
                    H34.append([x3b, x4b])

                wb = [[None, None], [None, None]]  # [c][kk]
                for c in range(2):
                    for kk in range(2):
                        wt = wp.tile(
                            [128, NU * 128], BF16, name="wb", tag=f"wb{c}{kk}"
                        )
                        nc.sync.dma_start(wt, wb16[2 * j + c, kk])
                        wb[c][kk] = wt
                wq1 = wq.tile([128, 2, NU * 128], FP8, name="wq1", tag="wq1")
                nc.sync.dma_start(wq1, wf8[j, 0])
                wq2 = wq.tile([128, 2, NU * 128], FP8, name="wq2", tag="wq2")
                nc.sync.dma_start(wq2, wf8[j, 1])

                def mm(u, which, start=False, stop=False):
                    us = slice(u * 128, (u + 1) * 128)
                    if which[0] == "q":  # fp8 DoubleRow: k=1 or 2
                        wqt, xq = (wq1, x1q) if which == "q1" else (wq2, x2q)
                        nc.tensor.matmul(
                            pacc[u], wqt[:, :, us], xq,
                            start=start, stop=stop, perf_mode=DR,
                        )
                    else:  # bf16: ("b", c, kk)
                        _, c, kk = which
                        nc.tensor.matmul(
                            pacc[u], wb[c][kk][:, us], H34[c][kk],
                            start=start, stop=stop,
                        )

                groups = [
                    ("b", 0, 0), ("b", 0, 1), "q1", "q2",
                    ("b", 1, 0), ("b", 1, 1),
                ]
                if j < NP - 1:
                    for gi, g in enumerate(groups):
                        for u in range(NU):
                            mm(u, g, start=(j == 0 and gi == 0))
                else:
                    # final pair u-major: each u finishes early so its
                    # Gelu + output DMA overlap the remaining matmuls
                    for u in range(NU):
                        for gi, g in enumerate(groups):
                            mm(u, g, stop=(gi == len(groups) - 1))
                        osb = op.tile([128, BL], F32, name="osb", tag="osb")
                        nc.scalar.activation(
                            osb, pacc[u], AF.Gelu, bias=bias_sb[:, u : u + 1]
                        )
                        nc.gpsimd.dma_start(
                            out_t[u * 128 : (u + 1) * 128, :], osb
                        )

    nc.compile()
    return nc


_NC_CACHE = None


def kernel(x, basis_weights, bias):
    global _NC_CACHE, LAST_EXEC_TIME_NS
    x = np.asarray(x, dtype=np.float32)
    W = np.asarray(basis_weights, dtype=np.float32)
    bias = np.asarray(bias, dtype=np.float32)

    # ---- host prep (layout only + constant folding of the x^0 term) ----
    xT = np.ascontiguousarray(x.T)  # (D, B)
    wb16 = np.ascontiguousarray(
        W[:, 3:5, :].reshape(ND, 128, 2, NU * 128).transpose(0, 2, 1, 3)
    ).astype(ml_dtypes.bfloat16)  # [ND, 2, 128, NU*128]
    wk12 = W[:, 1:3, :].reshape(NP, 2, 128, 2, NU * 128).transpose(0, 3, 2, 1, 4)
    wk12 = wk12 * np.array([1.0 / XS1, 1.0 / XS2], dtype=np.float32).reshape(
        1, 2, 1, 1, 1
    )
    wf8 = np.ascontiguousarray(np.clip(wk12, -240.0, 240.0)).astype(
        ml_dtypes.float8_e4m3
    )  # [NP, 2, 128, 2, NU*128]
    bias_total = (
        bias.astype(np.float64) + W[:, 0, :].astype(np.float64).sum(axis=0)
    ).astype(np.float32)
    bias2d = np.ascontiguousarray(bias_total.reshape(NU, 128).T)

    in_maps = []
    for i in range(NCORES):
        xt_i = np.ascontiguousarray(xT[:, i * BL : (i + 1) * BL])
        in_maps.append(
            {"xt": xt_i, "wb16": wb16, "wf8": wf8, "bias2d": bias2d}
        )

    if _NC_CACHE is None:
        _NC_CACHE = _build()
    nc = _NC_CACHE

    trace = bool(os.environ.get("KERNEL_TRACE"))
    res = run_bass_kernel_spmd(
        nc, in_maps, core_ids=list(range(NCORES)), trace=trace
    )
    LAST_EXEC_TIME_NS = res.exec_time_ns

    out = np.empty((B, U), dtype=np.float32)
    for i in range(NCORES):
        out[i * BL : (i + 1) * BL, :] = res.results[i]["out_t"].T
    return out


# revision 7
# speedup vs baseline: 3.2431x; 1.1021x over previous
"""KAN layer (polynomial basis) TRN2 kernel.

out = gelu(sum_{i,k} x[b,i]^k * W[i,k,j] + bias[j]),  exact gelu.
B=4096, D=1024, K=5, U=1024, fp32 I/O.

Strategy:
  - Data-parallel over batch: 8 cores x 512 rows each.
  - k=0 term (x^0=1) constant-folded on host into the bias.
  - Mixed precision chosen against the 2e-2 rel-err gate (offline sim:
    rel ~5.3e-3): k=1,2 terms in fp8e4 with DoubleRow perf mode (2
    contraction chunks per MM at 0.5 cyc/row), k=3,4 terms in bf16.
    Power-of-2 scales keep products exact: x*(1/4) vs W1*4, x^2*(1/16)
    vs W2*16.  All powers computed in fp32, rounded once.
  - Loop d-chunk-pair-outer / u-inner with all 8 PSUM banks as per-u
    accumulators; the last pair runs u-major so Gelu + output DMA
    pipeline with the final matmuls.
  - x/bias/out DMAs on the GpSimd queue, weight DMAs on the Sync
    queue (parallel issue), powers split across Scalar (Square) and
    DVE engines.
  - Output computed transposed ([U, B_local]) so the per-unit bias is a
    per-partition scalar, fused into the final Gelu activation; host
    transposes back during the gather.
"""

import os
import numpy as np
import ml_dtypes

from concourse import bacc
import concourse.mybir as mybir
import concourse.tile as tile
from concourse.bass_utils import run_bass_kernel_spmd

F32 = mybir.dt.float32
BF16 = mybir.dt.bfloat16
FP8 = mybir.dt.float8e4
AF = mybir.ActivationFunctionType
DR = mybir.MatmulPerfMode.DoubleRow

NCORES = 8
B, D, K, U = 4096, 1024, 5, 1024
BL = B // NCORES  # 512 batch rows per core
ND = D // 128  # 8 d chunks
NP = ND // 2  # 4 d-chunk pairs
NU = U // 128  # 8 u chunks

XS1 = 0.25      # x^1 fp8 pre-scale (W1 scaled by 1/XS1)
XS2 = 0.0625    # x^2 fp8 pre-scale (W2 scaled by 1/XS2)

LAST_EXEC_TIME_NS = None


def _build():
    nc = bacc.Bacc("TRN2", target_bir_lowering=False, debug=False)
    xt = nc.dram_tensor("xt", [D, BL], F32, kind="ExternalInput").ap()
    # wb16[d, kk, p, m] = W[d*128+p, kk+3, m]
    wb16 = nc.dram_tensor(
        "wb16", [ND, 2, 128, NU * 128], BF16, kind="ExternalInput"
    ).ap()
    # wf8[j, kk, p, c, m] = W[(2j+c)*128+p, kk+1, m] / XS_{kk+1}
    wf8 = nc.dram_tensor(
        "wf8", [NP, 2, 128, 2, NU * 128], FP8, kind="ExternalInput"
    ).ap()
    bias2d = nc.dram_tensor("bias2d", [128, NU], F32, kind="ExternalInput").ap()
    out_t = nc.dram_tensor("out_t", [U, BL], F32, kind="ExternalOutput").ap()

    with tile.TileContext(nc) as tc:
        with (
            tc.tile_pool(name="const", bufs=1) as constp,
            tc.tile_pool(name="xp", bufs=3) as xp,
            tc.tile_pool(name="pw", bufs=3) as pw,
            tc.tile_pool(name="qp", bufs=2) as qp,
            tc.tile_pool(name="wp", bufs=3) as wp,
            tc.tile_pool(name="wq", bufs=2) as wq,
            tc.tile_pool(name="op", bufs=4) as op,
            tc.tile_pool(name="ps", bufs=1, space="PSUM") as ps,
        ):
            bias_sb = constp.tile([128, NU], F32, name="bias_sb")

            # 8 persistent PSUM accumulators, one bank per u chunk.
            pacc = [ps.tile([128, BL], F32, name=f"pacc{u}") for u in range(NU)]

            # HAM warm-up: ~3.4us of dummy matmuls (zeroed operand, each a
            # complete start/stop group into pacc[0], later reset by the
            # real first accumulation) while the prologue DMAs stream, so
            # the PE clock gate is already 8/8 when the real stream starts.
            warm = constp.tile([128, BL], BF16, name="warm")
            nc.vector.memset(warm, 0)
            for _ in range(16):
                nc.tensor.matmul(
                    pacc[0], warm[:, 0:128], warm, start=True, stop=True
                )

            # x chunk DMAs for the first pair go out first on the Sync
            # queue so their transfers aren't queued behind the weight
            # transfers on the shared DMA path.
            xf_head = []
            for c in range(2):
                xf = xp.tile([128, BL], F32, name="xf", tag="xf")
                nc.sync.dma_start(xf, xt[c * 128 : (c + 1) * 128, :])
                xf_head.append(xf)

            for j in range(NP):
                if j == 1:
                    # bias transfer is tiny; keep it off both the prologue
                    # and epilogue DMA hot paths
                    nc.gpsimd.dma_start(bias_sb, bias2d)
                x1q = qp.tile([128, 2, BL], FP8, name="x1q", tag="x1q")
                x2q = qp.tile([128, 2, BL], FP8, name="x2q", tag="x2q")
                H34 = []  # [c][kk] bf16 tiles
                for c in range(2):
                    d = 2 * j + c
                    if j == 0:
                        xf = xf_head[c]
                    else:
                        xf = xp.tile([128, BL], F32, name="xf", tag="xf")
                        nc.gpsimd.dma_start(xf, xt[d * 128 : (d + 1) * 128, :])
                    x2f = xp.tile([128, BL], F32, name="x2f", tag="x2f")
                    nc.scalar.activation(x2f, xf, AF.Square)
                    nc.vector.tensor_scalar_mul(x1q[:, c], xf, XS1)
                    nc.scalar.activation(x2q[:, c], xf, AF.Square, scale=0.25)
                    x3b = pw.tile([128, BL], BF16, name="x3b", tag="x3b")
                    nc.vector.tensor_mul(out=x3b, in0=x2f, in1=xf)
                    x4b = pw.tile([128, BL], BF16, name="x4b", tag="x4b")
                    nc.scalar.activation(x4b, x2f, AF.Square)
                    H34.append([x3b, x4b])

                # kick weight DMAs in the order the MM groups consume them
                wb = [[None, None], [None, None]]  # [c][kk]
                for kk in range(2):
                    wt = wp.tile([128, NU * 128], BF16, name="wb", tag=f"wb0{kk}")
                    nc.sync.dma_start(wt, wb16[2 * j, kk])
                    wb[0][kk] = wt
                wq1 = wq.tile([128, 2, NU * 128], FP8, name="wq1", tag="wq1")
                nc.sync.dma_start(wq1, wf8[j, 0])
                wq2 = wq.tile([128, 2, NU * 128], FP8, name="wq2", tag="wq2")
                nc.sync.dma_start(wq2, wf8[j, 1])
                for kk in range(2):
                    wt = wp.tile([128, NU * 128], BF16, name="wb", tag=f"wb1{kk}")
                    nc.sync.dma_start(wt, wb16[2 * j + 1, kk])
                    wb[1][kk] = wt

                def mm(u, which, start=False, stop=False):
                    us = slice(u * 128, (u + 1) * 128)
                    if which[0] == "q":  # fp8 DoubleRow: k=1 or 2
                        wqt, xq = (wq1, x1q) if which == "q1" else (wq2, x2q)
                        nc.tensor.matmul(
                            pacc[u], wqt[:, :, us], xq,
                            start=start, stop=stop, perf_mode=DR,
                        )
                    else:  # bf16: ("b", c, kk)
                        _, c, kk = which
                        nc.tensor.matmul(
                            pacc[u], wb[c][kk][:, us], H34[c][kk],
                            start=start, stop=stop,
                        )

                groups = [
                    ("b", 0, 0), ("b", 0, 1), "q1", "q2",
                    ("b", 1, 0), ("b", 1, 1),
                ]
                if j < NP - 1:
                    for gi, g in enumerate(groups):
                        for u in range(NU):
                            mm(u, g, start=(j == 0 and gi == 0))
                else:
                    # final pair u-major: each u finishes early so its
                    # Gelu + output DMA overlap the remaining matmuls
                    for u in range(NU):
                        for gi, g in enumerate(groups):
                            mm(u, g, stop=(gi == len(groups) - 1))
                        osb = op.tile([128, BL], F32, name="osb", tag="osb")
                        nc.scalar.activation(
                            osb, pacc[u], AF.Gelu, bias=bias_sb[:, u : u + 1]
                        )
                        nc.gpsimd.dma_start(
                            out_t[u * 128 : (u + 1) * 128, :], osb
                        )

    nc.compile()
    return nc


_NC_CACHE = None


def kernel(x, basis_weights, bias):
    global _NC_CACHE, LAST_EXEC_TIME_NS
    x = np.asarray(x, dtype=np.float32)
    W = np.asarray(basis_weights, dtype=np.float32)
    bias = np.asarray(bias, dtype=np.float32)

    # ---- host prep (layout only + constant folding of the x^0 term) ----
    xT = np.ascontiguousarray(x.T)  # (D, B)
    wb16 = np.ascontiguousarray(
        W[:, 3:5, :].reshape(ND, 128, 2, NU * 128).transpose(0, 2, 1, 3)
    ).astype(ml_dtypes.bfloat16)  # [ND, 2, 128, NU*128]
    wk12 = W[:, 1:3, :].reshape(NP, 2, 128, 2, NU * 128).transpose(0, 3, 2, 1, 4)
    wk12 = wk12 * np.array([1.0 / XS1, 1.0 / XS2], dtype=np.float32).reshape(
        1, 2, 1, 1, 1
    )
    wf8 = np.ascontiguousarray(np.clip(wk12, -240.0, 240.0)).astype(
        ml_dtypes.float8_e4m3
    )  # [NP, 2, 128, 2, NU*128]
    bias_total = (
        bias.astype(np.float64) + W[:, 0, :].astype(np.float64).sum(axis=0)
    ).astype(np.float32)
    bias2d = np.ascontiguousarray(bias_total.reshape(NU, 128).T)

    in_maps = []
    for i in range(NCORES):
        xt_i = np.ascontiguousarray(xT[:, i * BL : (i + 1) * BL])
        in_maps.append(
            {"xt": xt_i, "wb16": wb16, "wf8": wf8, "bias2d": bias2d}
        )

    if _NC_CACHE is None:
        _NC_CACHE = _build()
    nc = _NC_CACHE

    trace = bool(os.environ.get("KERNEL_TRACE"))
    res = run_bass_kernel_spmd(
        nc, in_maps, core_ids=list(range(NCORES)), trace=trace
    )
    LAST_EXEC_TIME_NS = res.exec_time_ns

    out = np.empty((B, U), dtype=np.float32)
    for i in range(NCORES):
        out[i * BL : (i + 1) * BL, :] = res.results[i]["out_t"].T
    return out


# revision 8
# speedup vs baseline: 3.5132x; 1.0833x over previous
"""KAN layer (polynomial basis) TRN2 kernel.

out = gelu(sum_{i,k} x[b,i]^k * W[i,k,j] + bias[j]),  exact gelu.
B=4096, D=1024, K=5, U=1024, fp32 I/O.

Strategy:
  - Data-parallel over batch: 8 cores x 512 rows each.
  - k=0 term (x^0=1) constant-folded on host into the bias.
  - Mixed precision chosen against the 2e-2 rel-err gate (offline sim
    of the exact device arithmetic: rel ~1.43e-2): k=1,2,3 terms in
    fp8e4 with DoubleRow perf mode (2 contraction chunks per MM at
    0.5 cyc/row -> 2x bf16 rate), k=4 term in bf16.  Power-of-2
    scales keep products exact: x/4 vs 4*W1, x^2/16 vs 16*W2,
    x^3/32 vs 32*W3.  All powers computed in fp32, rounded once.
  - Loop d-chunk-pair-outer / u-inner with all 8 PSUM banks as per-u
    accumulators; the last pair runs u-major so Gelu + output DMA
    pipeline with the final matmuls.
  - First x-chunk DMAs kicked ahead of the weight stream (shared DMA
    path is bandwidth-bound at ~640ns per 256KB transfer), bias DMA
    deferred off the hot paths; x/out DMAs on the GpSimd queue,
    weights on the Sync queue.
  - ~3us of dummy warm-up matmuls during the DMA prologue flip the PE
    HAM clock gate to 8/8 before the real stream starts.
  - Output computed transposed ([U, B_local]) so the per-unit bias is a
    per-partition scalar, fused into the final Gelu activation; host
    transposes back during the gather.
"""

import os
import numpy as np
import ml_dtypes

from concourse import bacc
import concourse.mybir as mybir
import concourse.tile as tile
from concourse.bass_utils import run_bass_kernel_spmd

F32 = mybir.dt.float32
BF16 = mybir.dt.bfloat16
FP8 = mybir.dt.float8e4
AF = mybir.ActivationFunctionType
DR = mybir.MatmulPerfMode.DoubleRow
MUL = mybir.AluOpType.mult

NCORES = 8
B, D, K, U = 4096, 1024, 5, 1024
BL = B // NCORES  # 512 batch rows per core
ND = D // 128  # 8 d chunks
NP = ND // 2  # 4 d-chunk pairs
NU = U // 128  # 8 u chunks

XS = [0.25, 0.0625, 0.03125]  # fp8 pre-scales for x^1, x^2, x^3

LAST_EXEC_TIME_NS = None


def _build():
    nc = bacc.Bacc("TRN2", target_bir_lowering=False, debug=False)
    xt = nc.dram_tensor("xt", [D, BL], F32, kind="ExternalInput").ap()
    # wb16[d, p, m] = W[d*128+p, 4, m]
    wb16 = nc.dram_tensor(
        "wb16", [ND, 128, NU * 128], BF16, kind="ExternalInput"
    ).ap()
    # wf8[j, kk, p, c, m] = W[(2j+c)*128+p, kk+1, m] / XS[kk]
    wf8 = nc.dram_tensor(
        "wf8", [NP, 3, 128, 2, NU * 128], FP8, kind="ExternalInput"
    ).ap()
    bias2d = nc.dram_tensor("bias2d", [128, NU], F32, kind="ExternalInput").ap()
    out_t = nc.dram_tensor("out_t", [U, BL], F32, kind="ExternalOutput").ap()

    with tile.TileContext(nc) as tc:
        with (
            tc.tile_pool(name="const", bufs=1) as constp,
            tc.tile_pool(name="xp", bufs=3) as xp,
            tc.tile_pool(name="pw", bufs=3) as pw,
            tc.tile_pool(name="qp", bufs=2) as qp,
            tc.tile_pool(name="wp", bufs=3) as wp,
            tc.tile_pool(name="wq", bufs=2) as wq,
            tc.tile_pool(name="op", bufs=4) as op,
            tc.tile_pool(name="ps", bufs=1, space="PSUM") as ps,
        ):
            bias_sb = constp.tile([128, NU], F32, name="bias_sb")

            # 8 persistent PSUM accumulators, one bank per u chunk.
            pacc = [ps.tile([128, BL], F32, name=f"pacc{u}") for u in range(NU)]

            # HAM warm-up: ~3us of dummy matmuls (zeroed operand, each a
            # complete start/stop group into pacc[0], later reset by the
            # real first accumulation) while the prologue DMAs stream, so
            # the PE clock gate is already 8/8 when the real stream starts.
            warm = constp.tile([128, BL], BF16, name="warm")
            nc.vector.memset(warm, 0)
            for _ in range(7):
                nc.tensor.matmul(
                    pacc[0], warm[:, 0:128], warm, start=True, stop=True
                )

            # x chunk DMAs for the first pair go out first on the Sync
            # queue so their transfers aren't queued behind the weight
            # transfers on the shared DMA path.
            xf_head = []
            for c in range(2):
                xf = xp.tile([128, BL], F32, name="xf", tag="xf")
                nc.sync.dma_start(xf, xt[c * 128 : (c + 1) * 128, :])
                xf_head.append(xf)

            for j in range(NP):
                if j == 1:
                    # bias transfer is tiny; keep it off both the prologue
                    # and epilogue DMA hot paths
                    nc.gpsimd.dma_start(bias_sb, bias2d)
                xq = [
                    qp.tile([128, 2, BL], FP8, name=f"x{k}q", tag=f"x{k}q")
                    for k in (1, 2, 3)
                ]
                x4 = []  # bf16 x^4 per chunk
                for c in range(2):
                    d = 2 * j + c
                    if j == 0:
                        xf = xf_head[c]
                    else:
                        xf = xp.tile([128, BL], F32, name="xf", tag="xf")
                        nc.gpsimd.dma_start(xf, xt[d * 128 : (d + 1) * 128, :])
                    x2f = xp.tile([128, BL], F32, name="x2f", tag="x2f")
                    nc.scalar.activation(x2f, xf, AF.Square)
                    nc.vector.tensor_scalar_mul(xq[0][:, c], xf, XS[0])
                    # x^2/16 = (x/4)^2 ; x^3/32 = (x^2/32)*x — one DVE op each
                    nc.vector.scalar_tensor_tensor(
                        xq[1][:, c], xf, XS[1], xf, op0=MUL, op1=MUL
                    )
                    nc.vector.scalar_tensor_tensor(
                        xq[2][:, c], x2f, XS[2], xf, op0=MUL, op1=MUL
                    )
                    x4b = pw.tile([128, BL], BF16, name="x4b", tag="x4b")
                    nc.scalar.activation(x4b, x2f, AF.Square)
                    x4.append(x4b)

                # kick weight DMAs in the order the MM groups consume them
                wb = []
                wt = wp.tile([128, NU * 128], BF16, name="wb", tag="wb0")
                nc.sync.dma_start(wt, wb16[2 * j])
                wb.append(wt)
                wqt = []
                for kk in range(3):
                    wtq = wq.tile(
                        [128, 2, NU * 128], FP8, name=f"wq{kk}", tag=f"wq{kk}"
                    )
                    nc.sync.dma_start(wtq, wf8[j, kk])
                    wqt.append(wtq)
                wt = wp.tile([128, NU * 128], BF16, name="wb", tag="wb1")
                nc.sync.dma_start(wt, wb16[2 * j + 1])
                wb.append(wt)

                def mm(u, which, start=False, stop=False):
                    us = slice(u * 128, (u + 1) * 128)
                    if which[0] == "q":  # fp8 DoubleRow: k=1,2,3
                        kk = which[1]
                        nc.tensor.matmul(
                            pacc[u], wqt[kk][:, :, us], xq[kk],
                            start=start, stop=stop, perf_mode=DR,
                        )
                    else:  # bf16 k=4 for chunk c
                        c = which[1]
                        nc.tensor.matmul(
                            pacc[u], wb[c][:, us], x4[c],
                            start=start, stop=stop,
                        )

                groups = [("b", 0), ("q", 0), ("q", 1), ("q", 2), ("b", 1)]
                if j < NP - 1:
                    for gi, g in enumerate(groups):
                        for u in range(NU):
                            mm(u, g, start=(j == 0 and gi == 0))
                else:
                    # final pair u-major: each u finishes early so its
                    # Gelu + output DMA overlap the remaining matmuls
                    for u in range(NU):
                        for gi, g in enumerate(groups):
                            mm(u, g, stop=(gi == len(groups) - 1))
                        osb = op.tile([128, BL], F32, name="osb", tag="osb")
                        nc.scalar.activation(
                            osb, pacc[u], AF.Gelu, bias=bias_sb[:, u : u + 1]
                        )
                        nc.gpsimd.dma_start(
                            out_t[u * 128 : (u + 1) * 128, :], osb
                        )

    nc.compile()
    return nc


_NC_CACHE = None


def kernel(x, basis_weights, bias):
    global _NC_CACHE, LAST_EXEC_TIME_NS
    x = np.asarray(x, dtype=np.float32)
    W = np.asarray(basis_weights, dtype=np.float32)
    bias = np.asarray(bias, dtype=np.float32)

    # ---- host prep (layout only + constant folding of the x^0 term) ----
    xT = np.ascontiguousarray(x.T)  # (D, B)
    wb16 = np.ascontiguousarray(
        W[:, 4, :].reshape(ND, 128, NU * 128)
    ).astype(ml_dtypes.bfloat16)
    wk = W[:, 1:4, :].reshape(NP, 2, 128, 3, NU * 128).transpose(0, 3, 2, 1, 4)
    wk = wk * (1.0 / np.array(XS, dtype=np.float32)).reshape(1, 3, 1, 1, 1)
    wf8 = np.ascontiguousarray(np.clip(wk, -240.0, 240.0)).astype(
        ml_dtypes.float8_e4m3
    )  # [NP, 3, 128, 2, NU*128]
    bias_total = (
        bias.astype(np.float64) + W[:, 0, :].astype(np.float64).sum(axis=0)
    ).astype(np.float32)
    bias2d = np.ascontiguousarray(bias_total.reshape(NU, 128).T)

    in_maps = []
    for i in range(NCORES):
        xt_i = np.ascontiguousarray(xT[:, i * BL : (i + 1) * BL])
        in_maps.append(
            {"xt": xt_i, "wb16": wb16, "wf8": wf8, "bias2d": bias2d}
        )

    if _NC_CACHE is None:
        _NC_CACHE = _build()
    nc = _NC_CACHE

    trace = bool(os.environ.get("KERNEL_TRACE"))
    res = run_bass_kernel_spmd(
        nc, in_maps, core_ids=list(range(NCORES)), trace=trace
    )
    LAST_EXEC_TIME_NS = res.exec_time_ns

    out = np.empty((B, U), dtype=np.float32)
    for i in range(NCORES):
        out[i * BL : (i + 1) * BL, :] = res.results[i]["out_t"].T
    return out


# revision 10
# speedup vs baseline: 3.5737x; 1.0172x over previous
"""KAN layer (polynomial basis) TRN2 kernel.

out = gelu(sum_{i,k} x[b,i]^k * W[i,k,j] + bias[j]),  exact gelu.
B=4096, D=1024, K=5, U=1024, fp32 I/O.

Strategy:
  - Data-parallel over batch: 8 cores x 512 rows each.
  - k=0 term (x^0=1) constant-folded on host into the bias.
  - Mixed precision chosen against the 2e-2 rel-err gate (offline sim
    of the exact device arithmetic: rel ~1.43e-2): k=1,2,3 terms in
    fp8e4 with DoubleRow perf mode (2 contraction chunks per MM at
    0.5 cyc/row -> 2x bf16 rate), k=4 term in bf16.  Power-of-2
    scales keep products exact: x/4 vs 4*W1, x^2/16 vs 16*W2,
    x^3/32 vs 32*W3.  All powers computed in fp32, rounded once.
  - Loop d-chunk-pair-outer / u-inner with all 8 PSUM banks as per-u
    accumulators; the last pair runs u-major so Gelu + output DMA
    pipeline with the final matmuls.
  - First x-chunk DMAs kicked ahead of the weight stream (shared DMA
    path is bandwidth-bound at ~640ns per 256KB transfer), bias DMA
    deferred off the hot paths; x/out DMAs on the GpSimd queue,
    weights on the Sync queue.
  - ~3us of dummy warm-up matmuls during the DMA prologue flip the PE
    HAM clock gate to 8/8 before the real stream starts.
  - Output computed transposed ([U, B_local]) so the per-unit bias is a
    per-partition scalar, fused into the final Gelu activation; host
    transposes back during the gather.
"""

import os
import numpy as np
import ml_dtypes

from concourse import bacc
import concourse.mybir as mybir
import concourse.tile as tile
from concourse.bass_utils import run_bass_kernel_spmd

F32 = mybir.dt.float32
BF16 = mybir.dt.bfloat16
FP8 = mybir.dt.float8e4
AF = mybir.ActivationFunctionType
DR = mybir.MatmulPerfMode.DoubleRow
MUL = mybir.AluOpType.mult

NCORES = 8
B, D, K, U = 4096, 1024, 5, 1024
BL = B // NCORES  # 512 batch rows per core
ND = D // 128  # 8 d chunks
NP = ND // 2  # 4 d-chunk pairs
NU = U // 128  # 8 u chunks

XS = [0.25, 0.0625, 0.03125]  # fp8 pre-scales for x^1, x^2, x^3

LAST_EXEC_TIME_NS = None


def _build():
    nc = bacc.Bacc("TRN2", target_bir_lowering=False, debug=False)
    xt = nc.dram_tensor("xt", [D, BL], F32, kind="ExternalInput").ap()
    # wb16[d, p, m] = W[d*128+p, 4, m]
    wb16 = nc.dram_tensor(
        "wb16", [ND, 128, NU * 128], BF16, kind="ExternalInput"
    ).ap()
    # wf8[j, kk, p, c, m] = W[(2j+c)*128+p, kk+1, m] / XS[kk]
    wf8 = nc.dram_tensor(
        "wf8", [NP, 3, 128, 2, NU * 128], FP8, kind="ExternalInput"
    ).ap()
    bias2d = nc.dram_tensor("bias2d", [128, NU], F32, kind="ExternalInput").ap()
    out_t = nc.dram_tensor("out_t", [U, BL], F32, kind="ExternalOutput").ap()

    with tile.TileContext(nc) as tc:
        with (
            tc.tile_pool(name="const", bufs=1) as constp,
            tc.tile_pool(name="xp", bufs=3) as xp,
            tc.tile_pool(name="pw", bufs=3) as pw,
            tc.tile_pool(name="qp", bufs=2) as qp,
            tc.tile_pool(name="wp", bufs=3) as wp,
            tc.tile_pool(name="wq", bufs=2) as wq,
            tc.tile_pool(name="op", bufs=4) as op,
            tc.tile_pool(name="ps", bufs=1, space="PSUM") as ps,
        ):
            bias_sb = constp.tile([128, NU], F32, name="bias_sb")

            # 8 persistent PSUM accumulators, one bank per u chunk.
            pacc = [ps.tile([128, BL], F32, name=f"pacc{u}") for u in range(NU)]

            # HAM warm-up: ~3us of dummy matmuls (zeroed operand, each a
            # complete start/stop group into pacc[0], later reset by the
            # real first accumulation) while the prologue DMAs stream, so
            # the PE clock gate is already 8/8 when the real stream starts.
            warm = constp.tile([128, BL], BF16, name="warm")
            nc.vector.memset(warm, 0)
            for _ in range(5):
                nc.tensor.matmul(
                    pacc[0], warm[:, 0:128], warm, start=True, stop=True
                )

            for j in range(NP):
                if j == 1:
                    # bias transfer is tiny; keep it off both the prologue
                    # and epilogue DMA hot paths
                    nc.gpsimd.dma_start(bias_sb, bias2d)
                xq = [
                    qp.tile([128, 2, BL], FP8, name=f"x{k}q", tag=f"x{k}q")
                    for k in (1, 2, 3)
                ]
                # All input DMAs share the Sync queue, kicked in exact
                # consumption order: the DMA transfers share bandwidth
                # concurrently, so an eagerly-kicked later transfer slows
                # the one the next compute op is actually waiting on.
                xfs = []
                for c in range(2):
                    xf = xp.tile([128, BL], F32, name="xf", tag="xf")
                    xfs.append(xf)
                wb = [
                    wp.tile([128, NU * 128], BF16, name="wb", tag=f"wb{c}")
                    for c in range(2)
                ]
                wqt = [
                    wq.tile([128, 2, NU * 128], FP8, name=f"wq{kk}", tag=f"wq{kk}")
                    for kk in range(3)
                ]
                d0 = 2 * j
                nc.sync.dma_start(xfs[0], xt[d0 * 128 : (d0 + 1) * 128, :])
                nc.sync.dma_start(wb[0], wb16[d0])
                nc.sync.dma_start(xfs[1], xt[(d0 + 1) * 128 : (d0 + 2) * 128, :])
                for kk in range(3):
                    nc.sync.dma_start(wqt[kk], wf8[j, kk])
                nc.sync.dma_start(wb[1], wb16[d0 + 1])

                x4 = []  # bf16 x^4 per chunk
                for c in range(2):
                    xf = xfs[c]
                    x2f = xp.tile([128, BL], F32, name="x2f", tag="x2f")
                    nc.scalar.activation(x2f, xf, AF.Square)
                    nc.vector.tensor_scalar_mul(xq[0][:, c], xf, XS[0])
                    # x^2/16 = (x/4)^2 ; x^3/32 = (x^2/32)*x — one DVE op each
                    nc.vector.scalar_tensor_tensor(
                        xq[1][:, c], xf, XS[1], xf, op0=MUL, op1=MUL
                    )
                    nc.vector.scalar_tensor_tensor(
                        xq[2][:, c], x2f, XS[2], xf, op0=MUL, op1=MUL
                    )
                    x4b = pw.tile([128, BL], BF16, name="x4b", tag="x4b")
                    nc.scalar.activation(x4b, x2f, AF.Square)
                    x4.append(x4b)

                def mm(u, which, start=False, stop=False):
                    us = slice(u * 128, (u + 1) * 128)
                    if which[0] == "q":  # fp8 DoubleRow: k=1,2,3
                        kk = which[1]
                        nc.tensor.matmul(
                            pacc[u], wqt[kk][:, :, us], xq[kk],
                            start=start, stop=stop, perf_mode=DR,
                        )
                    else:  # bf16 k=4 for chunk c
                        c = which[1]
                        nc.tensor.matmul(
                            pacc[u], wb[c][:, us], x4[c],
                            start=start, stop=stop,
                        )

                groups = [("b", 0), ("q", 0), ("q", 1), ("q", 2), ("b", 1)]
                if j < NP - 1:
                    for gi, g in enumerate(groups):
                        for u in range(NU):
                            mm(u, g, start=(j == 0 and gi == 0))
                else:
                    # final pair u-major: each u finishes early so its
                    # Gelu + output DMA overlap the remaining matmuls
                    for u in range(NU):
                        for gi, g in enumerate(groups):
                            mm(u, g, stop=(gi == len(groups) - 1))
                        osb = op.tile([128, BL], F32, name="osb", tag="osb")
                        nc.scalar.activation(
                            osb, pacc[u], AF.Gelu, bias=bias_sb[:, u : u + 1]
                        )
                        nc.gpsimd.dma_start(
                            out_t[u * 128 : (u + 1) * 128, :], osb
                        )

    nc.compile()
    return nc


_NC_CACHE = None


def kernel(x, basis_weights, bias):
    global _NC_CACHE, LAST_EXEC_TIME_NS
    x = np.asarray(x, dtype=np.float32)
    W = np.asarray(basis_weights, dtype=np.float32)
    bias = np.asarray(bias, dtype=np.float32)

    # ---- host prep (layout only + constant folding of the x^0 term) ----
    xT = np.ascontiguousarray(x.T)  # (D, B)
    wb16 = np.ascontiguousarray(
        W[:, 4, :].reshape(ND, 128, NU * 128)
    ).astype(ml_dtypes.bfloat16)
    wk = W[:, 1:4, :].reshape(NP, 2, 128, 3, NU * 128).transpose(0, 3, 2, 1, 4)
    wk = wk * (1.0 / np.array(XS, dtype=np.float32)).reshape(1, 3, 1, 1, 1)
    wf8 = np.ascontiguousarray(np.clip(wk, -240.0, 240.0)).astype(
        ml_dtypes.float8_e4m3
    )  # [NP, 3, 128, 2, NU*128]
    bias_total = (
        bias.astype(np.float64) + W[:, 0, :].astype(np.float64).sum(axis=0)
    ).astype(np.float32)
    bias2d = np.ascontiguousarray(bias_total.reshape(NU, 128).T)

    in_maps = []
    for i in range(NCORES):
        xt_i = np.ascontiguousarray(xT[:, i * BL : (i + 1) * BL])
        in_maps.append(
            {"xt": xt_i, "wb16": wb16, "wf8": wf8, "bias2d": bias2d}
        )

    if _NC_CACHE is None:
        _NC_CACHE = _build()
    nc = _NC_CACHE

    trace = bool(os.environ.get("KERNEL_TRACE"))
    res = run_bass_kernel_spmd(
        nc, in_maps, core_ids=list(range(NCORES)), trace=trace
    )
    LAST_EXEC_TIME_NS = res.exec_time_ns

    out = np.empty((B, U), dtype=np.float32)
    for i in range(NCORES):
        out[i * BL : (i + 1) * BL, :] = res.results[i]["out_t"].T
    return out
